# revision 8
# baseline (speedup 1.0000x reference)
"""Trainium2 Bass kernel for LoraLinear:
    out = x @ W^T + 2.0 * (x @ A^T) @ B^T
    x: [4, 2048, 4096] f32, W: [4096, 4096], A: [64, 4096], B: [4096, 64]

Sharding across 8 NeuronCores: 4-way data-parallel over tokens x 2-way
tensor-parallel over out-features. Each core computes a [2048 x 2048]
output block. No collectives; the host scatters shards and gathers blocks.

Per-core device program (SPMD, same program on all 8 cores):
  - W^T shard ([4096 x 2048] bf16, 16.8 MB) is loaded once and kept
    resident in SBUF; A^T and (2B)^T shards are also resident.
  - Loop over 16 token tiles (128 tokens each): DMA the x^T column block
    ([128 x 4096] bf16, 1 MB, host pre-transposed so the transfer is a
    plain contiguous 2D copy), compute xa^T = A @ x^T ([64 x 128]) in
    PSUM, then for each of 4 out-feature tiles accumulate 32 k-tile
    matmuls (base) + 1 rank-64 matmul (lora) into a [128 x 512] PSUM
    tile, copy to SBUF, DMA to the output block.

Matmuls run in fp16 (inputs are host-cast; same PE rate as bf16 with 8x
finer mantissa); accumulation is fp32 in PSUM.
All DMAs are simple 2D transfers — HWDGE queue fanout for 3D shapes breaks
Tile's semaphore accounting on this stack (sim race detector confirms).
"""

import numpy as np

import concourse.mybir as mybir
import concourse.tile as tile
from concourse import bacc
from concourse.bass_utils import run_bass_kernel_spmd

# problem dims (hardcoded per harness contract)
B, S, D_IN, D_OUT, R = 4, 2048, 4096, 4096, 64
SCALING = 2.0

T_TOTAL = B * S  # 8192 tokens
DP, TP = 4, 2  # token-parallel x feature-parallel over 8 cores
T_CORE = T_TOTAL // DP  # 2048
O_CORE = D_OUT // TP  # 2048
K = D_IN  # 4096

P = 128  # SBUF partitions / matmul contraction tile
KT = K // P  # 32 k-tiles
TT = T_CORE // P  # 16 token tiles per core
NO = 512  # matmul moving free dim (one PSUM bank of fp32)
OT = O_CORE // NO  # 4 out-feature tiles per core

MM_DT = mybir.dt.float16  # fp16: 1 cyc/row on PE like bf16, 8x finer mantissa
MM_NP = np.float16
F32 = mybir.dt.float32

_NC_CACHE = {}


def _build_program():
    nc = bacc.Bacc()
    # xp[tt][p][kt*128+t] = x^T[kt*128+p, tt*128+t]  (host pre-arranged)
    xp = nc.declare_dram_parameter("xp", [TT, P, KT * P], MM_DT, isOutput=False)
    wt = nc.declare_dram_parameter("wt", [K, O_CORE], MM_DT, isOutput=False)
    # ap[p][kt*64+r] = A^T[kt*128+p, r]  (host pre-arranged)
    ap = nc.declare_dram_parameter("ap", [P, KT * R], MM_DT, isOutput=False)
    bt = nc.declare_dram_parameter("bt", [R, O_CORE], MM_DT, isOutput=False)
    out = nc.declare_dram_parameter("out", [T_CORE, O_CORE], F32, isOutput=True)

    with tile.TileContext(nc) as tc:
        with (
            tc.tile_pool(name="wres", bufs=1) as wres,
            tc.tile_pool(name="xin", bufs=3) as xin,
            tc.tile_pool(name="ostage", bufs=4) as ostage,
            tc.tile_pool(name="psacc", bufs=4, space="PSUM") as psacc,
            tc.tile_pool(name="psxa", bufs=2, space="PSUM") as psxa,
        ):
            # A^T and (2B)^T go first on the SP queue so phase 1 can start
            # within ~1us; the 32 W^T blocks (16.8 MB, ~47us) follow.
            atile = wres.tile([P, KT * R], MM_DT, name="atile")
            nc.sync.dma_start(out=atile[:], in_=ap[:])
            btile = wres.tile([R, O_CORE], MM_DT, name="btile")
            nc.sync.dma_start(out=btile[:], in_=bt[:])

            # resident W^T as 32 k-blocks side by side -> [128, 32*2048]
            wtile = wres.tile([P, KT * O_CORE], MM_DT, name="wtile")
            wt_r = wt[:].rearrange("(kt p) o -> kt p o", p=P)
            for k in range(KT):
                nc.sync.dma_start(
                    out=wtile[:, k * O_CORE : (k + 1) * O_CORE], in_=wt_r[k]
                )

            # all xa^T tiles stay resident: [64, 16*128] fp16
            xa_all = wres.tile([R, TT * P], MM_DT, name="xa_all")

            # phase 1 — while W^T streams in on the SP queue, stream x on the
            # ACT queue and compute xa^T = A @ x^T for every token tile
            for t in range(TT):
                xtile = xin.tile([P, KT * P], MM_DT, name="xtile")
                nc.scalar.dma_start(out=xtile[:], in_=xp[t])
                ps_xa = psxa.tile([R, P], F32, name="ps_xa")
                for k in range(KT):
                    nc.tensor.matmul(
                        ps_xa[:],
                        atile[:, k * R : (k + 1) * R],
                        xtile[:, k * P : (k + 1) * P],
                        start=(k == 0),
                        stop=(k == KT - 1),
                    )
                nc.vector.tensor_copy(xa_all[:, t * P : (t + 1) * P], ps_xa[:])

            # phase 2 — base matmuls (x streams a second time, hidden under
            # ~460us of PE work) + rank-64 lora update into the same PSUM
            for t in range(TT):
                xtile = xin.tile([P, KT * P], MM_DT, name="xtile")
                nc.scalar.dma_start(out=xtile[:], in_=xp[t])
                for o in range(OT):
                    ps = psacc.tile([P, NO], F32, name="ps")
                    for k in range(KT):
                        nc.tensor.matmul(
                            ps[:],
                            xtile[:, k * P : (k + 1) * P],
                            wtile[:, k * O_CORE + o * NO : k * O_CORE + o * NO + NO],
                            start=(k == 0),
                            stop=False,
                        )
                    nc.tensor.matmul(
                        ps[:],
                        xa_all[:, t * P : (t + 1) * P],
                        btile[:, o * NO : (o + 1) * NO],
                        start=False,
                        stop=True,
                    )
                    osb = ostage.tile([P, NO], F32, name="osb")
                    nc.vector.tensor_copy(osb[:], ps[:])
                    nc.sync.dma_start(
                        out=out[t * P : (t + 1) * P, o * NO : (o + 1) * NO],
                        in_=osb[:],
                    )
    return nc


def _get_program():
    if "nc" not in _NC_CACHE:
        nc = _build_program()
        nc.finalize()  # runs Bacc.compile(): reg alloc, event-sem wait splitting
        _NC_CACHE["nc"] = nc
    return _NC_CACHE["nc"]


def _prep_x_shard(xs):
    """[T_CORE, K] f32 -> [TT, P, KT*P] bf16 with xp[tt,p,kt*128+t] = xs[tt*128+t, kt*128+p]."""
    x4 = xs.reshape(TT, P, KT, P)  # [tt, t, kt, p]
    return np.ascontiguousarray(x4.transpose(0, 3, 2, 1)).astype(
        MM_NP
    ).reshape(TT, P, KT * P)


def _prep_in_maps(x, weight, lora_A, lora_B):
    xf = np.ascontiguousarray(x.reshape(T_TOTAL, K))

    # ap[p, kt*64+r] = A[r, kt*128+p]
    a3 = lora_A.reshape(R, KT, P)  # [r, kt, p]
    ap_host = np.ascontiguousarray(a3.transpose(2, 1, 0)).astype(
        MM_NP
    ).reshape(P, KT * R)

    xp_shards = [
        _prep_x_shard(xf[d * T_CORE : (d + 1) * T_CORE]) for d in range(DP)
    ]

    wt_shards, bt_shards = [], []
    for tp in range(TP):
        ws = weight[tp * O_CORE : (tp + 1) * O_CORE]
        wt_shards.append(np.ascontiguousarray(ws.T).astype(MM_NP))
        bs = (SCALING * lora_B[tp * O_CORE : (tp + 1) * O_CORE]).astype(np.float32)
        bt_shards.append(np.ascontiguousarray(bs.T).astype(MM_NP))

    in_maps = []
    for core in range(8):
        d, tp = core // TP, core % TP
        in_maps.append(
            {
                "xp": xp_shards[d],
                "wt": wt_shards[tp],
                "ap": ap_host,
                "bt": bt_shards[tp],
            }
        )
    return in_maps


def _gather(results):
    out = np.empty((T_TOTAL, D_OUT), dtype=np.float32)
    for core in range(8):
        d, tp = core // TP, core % TP
        out[d * T_CORE : (d + 1) * T_CORE, tp * O_CORE : (tp + 1) * O_CORE] = results[
            core
        ]["out"]
    return out.reshape(B, S, D_OUT)


def run(x, weight, lora_A, lora_B, trace=False):
    """Returns (output, BassKernelResults)."""
    nc = _get_program()
    in_maps = _prep_in_maps(
        np.asarray(x, dtype=np.float32),
        np.asarray(weight, dtype=np.float32),
        np.asarray(lora_A, dtype=np.float32),
        np.asarray(lora_B, dtype=np.float32),
    )
    res = run_bass_kernel_spmd(nc, in_maps, list(range(8)), trace=trace)
    return _gather(res.results), res


def kernel(x, weight, lora_A, lora_B):
    out, _ = run(x, weight, lora_A, lora_B, trace=False)
    return out


# revision 9
# speedup vs baseline: 1.0885x; 1.0885x over previous
"""Trainium2 Bass kernel for LoraLinear:
    out = x @ W^T + 2.0 * (x @ A^T) @ B^T
    x: [4, 2048, 4096] f32, W: [4096, 4096], A: [64, 4096], B: [4096, 64]

Sharding across 8 NeuronCores: 4-way data-parallel over tokens x 2-way
tensor-parallel over out-features. Each core computes a [2048 x 2048]
output block. No collectives; the host scatters shards and gathers blocks.

Per-core device program (SPMD, same program on all 8 cores):
  - W^T shard ([4096 x 2048] bf16, 16.8 MB) is loaded once and kept
    resident in SBUF; A^T and (2B)^T shards are also resident.
  - Loop over 16 token tiles (128 tokens each): DMA the x^T column block
    ([128 x 4096] bf16, 1 MB, host pre-transposed so the transfer is a
    plain contiguous 2D copy), compute xa^T = A @ x^T ([64 x 128]) in
    PSUM, then for each of 4 out-feature tiles accumulate 32 k-tile
    matmuls (base) + 1 rank-64 matmul (lora) into a [128 x 512] PSUM
    tile, copy to SBUF, DMA to the output block.

Matmuls run in fp16 (inputs are host-cast; same PE rate as bf16 with 8x
finer mantissa); accumulation is fp32 in PSUM.
All DMAs are simple 2D transfers — HWDGE queue fanout for 3D shapes breaks
Tile's semaphore accounting on this stack (sim race detector confirms).
"""

import numpy as np

import concourse.mybir as mybir
import concourse.tile as tile
from concourse import bacc
from concourse.bass_utils import run_bass_kernel_spmd

# problem dims (hardcoded per harness contract)
B, S, D_IN, D_OUT, R = 4, 2048, 4096, 4096, 64
SCALING = 2.0

T_TOTAL = B * S  # 8192 tokens
DP, TP = 4, 2  # token-parallel x feature-parallel over 8 cores
T_CORE = T_TOTAL // DP  # 2048
O_CORE = D_OUT // TP  # 2048
K = D_IN  # 4096

P = 128  # SBUF partitions / matmul contraction tile
KT = K // P  # 32 k-tiles
TT = T_CORE // P  # 16 token tiles per core
NO = 512  # matmul moving free dim (one PSUM bank of fp32)
OT = O_CORE // NO  # 4 out-feature tiles per core

MM_DT = mybir.dt.float16  # fp16: 1 cyc/row on PE like bf16, 8x finer mantissa
MM_NP = np.float16
F32 = mybir.dt.float32

_NC_CACHE = {}


def _build_program():
    nc = bacc.Bacc()
    # xp[tt][p][kt*128+t] = x^T[kt*128+p, tt*128+t]  (host pre-arranged)
    xp = nc.declare_dram_parameter("xp", [TT, P, KT * P], MM_DT, isOutput=False)
    wt = nc.declare_dram_parameter("wt", [K, O_CORE], MM_DT, isOutput=False)
    # ap[p][kt*64+r] = A^T[kt*128+p, r]  (host pre-arranged)
    ap = nc.declare_dram_parameter("ap", [P, KT * R], MM_DT, isOutput=False)
    bt = nc.declare_dram_parameter("bt", [R, O_CORE], MM_DT, isOutput=False)
    out = nc.declare_dram_parameter("out", [T_CORE, O_CORE], F32, isOutput=True)

    STARTUP_T = 2  # token tiles whose base matmuls run k-outer during W load
    STARTUP_O = 3  # o-tiles per startup token tile (6 PSUM banks + 2 for xa)

    with tile.TileContext(nc) as tc:
        with (
            tc.tile_pool(name="wres", bufs=1) as wres,
            tc.tile_pool(name="xin", bufs=4) as xin,
            tc.tile_pool(name="xa", bufs=3) as xapool,
            tc.tile_pool(name="ostage", bufs=3) as ostage,
            tc.tile_pool(name="psacc", bufs=6, space="PSUM") as psacc,
            tc.tile_pool(name="psxa", bufs=2, space="PSUM") as psxa,
        ):
            # A^T and (2B)^T go first on the SP queue (~1us), then the 32
            # W^T blocks (16.8 MB, ~47us at HBM rate).
            atile = wres.tile([P, KT * R], MM_DT, name="atile")
            nc.sync.dma_start(out=atile[:], in_=ap[:])
            btile = wres.tile([R, O_CORE], MM_DT, name="btile")
            nc.sync.dma_start(out=btile[:], in_=bt[:])

            # resident W^T as 32 k-blocks side by side -> [128, 32*2048]
            wtile = wres.tile([P, KT * O_CORE], MM_DT, name="wtile")
            wt_r = wt[:].rearrange("(kt p) o -> kt p o", p=P)
            for k in range(KT):
                nc.sync.dma_start(
                    out=wtile[:, k * O_CORE : (k + 1) * O_CORE], in_=wt_r[k]
                )

            xtiles, xa_sbs = {}, {}

            def load_x(t):
                xt_ = xin.tile([P, KT * P], MM_DT, name="xtile", tag="xtile")
                nc.scalar.dma_start(out=xt_[:], in_=xp[t])
                xtiles[t] = xt_

            def compute_xa(t):
                ps_xa = psxa.tile([R, P], F32, name="ps_xa")
                for k in range(KT):
                    nc.tensor.matmul(
                        ps_xa[:],
                        atile[:, k * R : (k + 1) * R],
                        xtiles[t][:, k * P : (k + 1) * P],
                        start=(k == 0),
                        stop=(k == KT - 1),
                    )
                xa_sb = xapool.tile([R, P], MM_DT, name="xa_sb", tag="xa_sb")
                nc.vector.tensor_copy(xa_sb[:], ps_xa[:])
                xa_sbs[t] = xa_sb

            def finish_tile(t, o, ps):
                """lora accumulate + copy out + store (releases the PSUM slot)."""
                nc.tensor.matmul(
                    ps[:],
                    xa_sbs[t][:],
                    btile[:, o * NO : (o + 1) * NO],
                    start=False,
                    stop=True,
                )
                osb = ostage.tile([P, NO], F32, name="osb")
                nc.vector.tensor_copy(osb[:], ps[:])
                nc.sync.dma_start(
                    out=out[t * P : (t + 1) * P, o * NO : (o + 1) * NO],
                    in_=osb[:],
                )

            def base_pass(t, o):
                ps = psacc.tile([P, NO], F32, name="ps", tag="ps")
                for k in range(KT):
                    nc.tensor.matmul(
                        ps[:],
                        xtiles[t][:, k * P : (k + 1) * P],
                        wtile[:, k * O_CORE + o * NO : k * O_CORE + o * NO + NO],
                        start=(k == 0),
                        stop=False,
                    )
                finish_tile(t, o, ps)

            # --- startup: fill the W-load window with W-independent xa work,
            # then consume W blocks AS THEY ARRIVE (k-outer over a group of
            # STARTUP_T x STARTUP_O PSUM accumulators, ~1.3us PE per block vs
            # ~1.5us arrival), instead of stalling until the full W is in.
            for t in range(STARTUP_T):
                load_x(t)
                compute_xa(t)
            startup_ps = {
                (t, o): psacc.tile([P, NO], F32, name="ps", tag="ps")
                for t in range(STARTUP_T)
                for o in range(STARTUP_O)
            }
            for k in range(KT):
                for t in range(STARTUP_T):
                    for o in range(STARTUP_O):
                        nc.tensor.matmul(
                            startup_ps[t, o][:],
                            xtiles[t][:, k * P : (k + 1) * P],
                            wtile[:, k * O_CORE + o * NO : k * O_CORE + o * NO + NO],
                            start=(k == 0),
                            stop=False,
                        )
            for t in range(STARTUP_T):
                for o in range(STARTUP_O):
                    finish_tile(t, o, startup_ps[t, o])
            for t in range(STARTUP_T):
                for o in range(STARTUP_O, OT):
                    base_pass(t, o)

            # --- steady state ---
            for t in range(STARTUP_T, TT):
                load_x(t)
                compute_xa(t)
                for o in range(OT):
                    base_pass(t, o)
    return nc


def _get_program():
    if "nc" not in _NC_CACHE:
        nc = _build_program()
        nc.finalize()  # runs Bacc.compile(): reg alloc, event-sem wait splitting
        _NC_CACHE["nc"] = nc
    return _NC_CACHE["nc"]


def _prep_x_shard(xs):
    """[T_CORE, K] f32 -> [TT, P, KT*P] bf16 with xp[tt,p,kt*128+t] = xs[tt*128+t, kt*128+p]."""
    x4 = xs.reshape(TT, P, KT, P)  # [tt, t, kt, p]
    return np.ascontiguousarray(x4.transpose(0, 3, 2, 1)).astype(
        MM_NP
    ).reshape(TT, P, KT * P)


def _prep_in_maps(x, weight, lora_A, lora_B):
    xf = np.ascontiguousarray(x.reshape(T_TOTAL, K))

    # ap[p, kt*64+r] = A[r, kt*128+p]
    a3 = lora_A.reshape(R, KT, P)  # [r, kt, p]
    ap_host = np.ascontiguousarray(a3.transpose(2, 1, 0)).astype(
        MM_NP
    ).reshape(P, KT * R)

    xp_shards = [
        _prep_x_shard(xf[d * T_CORE : (d + 1) * T_CORE]) for d in range(DP)
    ]

    wt_shards, bt_shards = [], []
    for tp in range(TP):
        ws = weight[tp * O_CORE : (tp + 1) * O_CORE]
        wt_shards.append(np.ascontiguousarray(ws.T).astype(MM_NP))
        bs = (SCALING * lora_B[tp * O_CORE : (tp + 1) * O_CORE]).astype(np.float32)
        bt_shards.append(np.ascontiguousarray(bs.T).astype(MM_NP))

    in_maps = []
    for core in range(8):
        d, tp = core // TP, core % TP
        in_maps.append(
            {
                "xp": xp_shards[d],
                "wt": wt_shards[tp],
                "ap": ap_host,
                "bt": bt_shards[tp],
            }
        )
    return in_maps


def _gather(results):
    out = np.empty((T_TOTAL, D_OUT), dtype=np.float32)
    for core in range(8):
        d, tp = core // TP, core % TP
        out[d * T_CORE : (d + 1) * T_CORE, tp * O_CORE : (tp + 1) * O_CORE] = results[
            core
        ]["out"]
    return out.reshape(B, S, D_OUT)


def run(x, weight, lora_A, lora_B, trace=False):
    """Returns (output, BassKernelResults)."""
    nc = _get_program()
    in_maps = _prep_in_maps(
        np.asarray(x, dtype=np.float32),
        np.asarray(weight, dtype=np.float32),
        np.asarray(lora_A, dtype=np.float32),
        np.asarray(lora_B, dtype=np.float32),
    )
    res = run_bass_kernel_spmd(nc, in_maps, list(range(8)), trace=trace)
    return _gather(res.results), res


def kernel(x, weight, lora_A, lora_B):
    out, _ = run(x, weight, lora_A, lora_B, trace=False)
    return out
